# revision 25
# baseline (speedup 1.0000x reference)
"""Trainium2 Bass kernel for nn_EosLayer (gated linear-attention recurrence).

Sharding: 8 cores = 4 batches x 2 sequence halves. Each core processes
T = 256 (warmup) + 2048 (output) timesteps of one batch. The warmup window
replaces cross-core state passing: the per-(k,d) decay o < 0.97 makes
history older than 256 steps contribute < 3e-3 relative, well under the
bf16 noise floor of this implementation.

Per-core layout is d-major (d on partitions, time on the free dim).
Chunk grid: [256 (pure warmup), 512, 512, 512, 512].

Engine assignment (driven by the TimelineSim cost model):
  - PE: i/e/s projections (f32r), LN-stat ones-matmuls, output GEMM with
    rank-1 corrections for -mu*h and beta@W_out (all bf16).
  - DVE: all elementwise tensor-tensor work in bf16 (z = e*i, w = s*m,
    k-tree-reduction) at the 2x 16-bit rate, plus LN row math.
  - Pool: the per-(k,d-tile) hardware scans (its best cost ratio op);
    a few scan sets go to DVE for balance.
  - Act: PSUM evacuations (downcast to bf16), y^2 squares, final scale.
  - DMA: e/s partition-replication via DRAM bounce (bf16 halves traffic).

The scan keeps fp32 internal state (hardware guarantee), reads bf16 z,
writes bf16 m; decay columns stay f32 (bf16 decay would bias every step).
Chunk carries are bf16 views of the previous chunk's scan output used
directly as the `initial` operand.
"""

import numpy as np
import ml_dtypes

D = 512
K = 8
TAU = 16.0
EPS = 1e-5
B = 4
N = 4096
H = N // 2          # output rows per core
W = 256             # warmup rows
T = W + H           # 2304 rows processed per core
CHUNKS = [256, 512, 512, 512, 512]   # chunk 0 is pure warmup
NDO = 4             # d-tiles of 128 partitions
P = 128
TCMAX = 512

_CACHE = {}


def _build():
    import concourse.bass as bass
    import concourse.mybir as mybir
    import concourse.tile as tile
    from concourse.bacc import Bacc

    f32 = mybir.dt.float32
    f32r = mybir.dt.float32r
    bf16 = mybir.dt.bfloat16
    AF = mybir.ActivationFunctionType
    OP = mybir.AluOpType

    nc = Bacc("TRN2", target_bir_lowering=False, debug=False,
              enable_asserts=False, num_devices=8)

    # per-core input (pre-transposed x slice), shared weight/const inputs
    xt = nc.dram_tensor("xt", (D, T), f32r, kind="ExternalInput")
    wi = nc.dram_tensor("wi", (D, D), f32r, kind="ExternalInput")
    wes = nc.dram_tensor("wes", (D, 2 * K), f32r, kind="ExternalInput")
    oc = nc.dram_tensor("oc", (D, K), f32, kind="ExternalInput")      # o.T
    wo = nc.dram_tensor("wo", (D, D), bf16, kind="ExternalInput")     # gamma-folded
    hrow = nc.dram_tensor("hrow", (1, D), bf16, kind="ExternalInput")  # colsum(wo)
    bowr = nc.dram_tensor("bowr", (1, D), bf16, kind="ExternalInput")  # beta @ W_out
    yout = nc.dram_tensor("yout", (H, D), f32, kind="ExternalOutput")

    NCHUNK = len(CHUNKS)
    coff = [sum(CHUNKS[:i]) for i in range(NCHUNK)]   # chunk start columns

    with tile.TileContext(nc) as tc:
        with tc.tile_pool(name="const", bufs=1) as cst, \
             tc.tile_pool(name="work", bufs=2) as wk, \
             tc.tile_pool(name="xtp", bufs=1) as xtp, \
             tc.tile_pool(name="big", bufs=2) as big, \
             tc.tile_pool(name="zmp", bufs=3) as zmp, \
             tc.tile_pool(name="pmm", bufs=4, space="PSUM") as pmm, \
             tc.tile_pool(name="pes", bufs=1, space="PSUM") as pes, \
             tc.tile_pool(name="pg", bufs=1, space="PSUM") as pg, \
             tc.tile_pool(name="dr", bufs=2, space="DRAM") as dr:

            # ---- constants (loaded once) ----
            wi_sb = [cst.tile([P, D], f32r, tag=f"wi{t}", name=f"wi{t}") for t in range(NDO)]
            wes_sb = [cst.tile([P, 2 * K], f32r, tag=f"wes{t}", name=f"wes{t}") for t in range(NDO)]
            oc_sb = [cst.tile([P, K], f32, tag=f"oc{t}", name=f"oc{t}") for t in range(NDO)]
            wo_sb = [cst.tile([P, D], bf16, tag=f"wo{t}", name=f"wo{t}") for t in range(NDO)]
            # wes/wi feed chunk 0's projections immediately (SP queue, ahead
            # of the xt stream); oc/wo/h/bow are needed later and issue from
            # the Act queue so they don't serialize the SP queue.
            for t in range(NDO):
                sl = slice(t * P, (t + 1) * P)
                nc.sync.dma_start(out=wes_sb[t], in_=wes[sl, :])
                nc.sync.dma_start(out=wi_sb[t], in_=wi[sl, :])
            for t in range(NDO):
                sl = slice(t * P, (t + 1) * P)
                nc.gpsimd.dma_start(out=oc_sb[t], in_=oc[sl, :])
                nc.gpsimd.dma_start(out=wo_sb[t], in_=wo[sl, :])
            h_sb = cst.tile([1, D], bf16, tag="h", name="h")
            nc.gpsimd.dma_start(out=h_sb, in_=hrow[:, :])
            bow_sb = cst.tile([1, D], bf16, tag="bow", name="bow")
            nc.gpsimd.dma_start(out=bow_sb, in_=bowr[:, :])
            ones_sb = cst.tile([P, 1], bf16, tag="ones", name="ones")
            nc.vector.memset(ones_sb, 1.0)
            onerow = cst.tile([1, P], bf16, tag="onerow", name="onerow")
            nc.vector.memset(onerow, 1.0)
            eps_sb = cst.tile([P, 1], f32, tag="eps", name="eps")
            nc.vector.memset(eps_sb, EPS)

            # per-d-tile scan carries (extracted before the in-place readout)
            carry = [cst.tile([P, K], f32, tag=f"carry{t}", name=f"carry{t}")
                     for t in range(NDO)]

            def front_end(c):
                """Steps 1-4 of chunk c: x load, projections, evacuations,
                e/s partition-replication. Emitted one chunk ahead so PE's
                projections are queued before the previous chunk's LN/GEMM
                tail (avoids head-of-line chunk serialization)."""
                TC = CHUNKS[c]
                csl = slice(coff[c], coff[c] + TC)
                with tc.high_priority():
                    xt_sb = [xtp.tile([P, TCMAX], f32r, tag=f"xt{t}",
                                      name=f"xt{t}") for t in range(NDO)]
                    for t in range(NDO):
                        nc.sync.dma_start(out=xt_sb[t][:, 0:TC],
                                          in_=xt[t * P:(t + 1) * P, csl])
                    es_ps = pes.tile([2 * K, TCMAX], f32, tag="esps", name="esps")
                    for kt in range(NDO):
                        nc.tensor.matmul(es_ps[:, 0:TC], wes_sb[kt][:, :],
                                         xt_sb[kt][:, 0:TC],
                                         start=(kt == 0), stop=(kt == NDO - 1))
                    es_sb = wk.tile([2 * K, TCMAX], bf16, tag="es", name="es")
                    nc.scalar.copy(out=es_sb[:, 0:TC], in_=es_ps[:, 0:TC])
                    es_d = dr.tile([2 * K, TCMAX], bf16, tag="esd", name="esd")
                    nc.scalar.dma_start(out=es_d[:, 0:TC], in_=es_sb[:, 0:TC])
                    e_rep = big.tile([P, K * TCMAX], bf16, tag="erep", name="erep")
                    esrc = bass.AP(tensor=es_d.tensor, offset=es_d.offset,
                                   ap=[[0, P], [TCMAX, K], [1, TC]])
                    nc.scalar.dma_start(
                        out=e_rep[:, :].rearrange("p (k t) -> p k t", k=K)[:, :, 0:TC],
                        in_=esrc)
                    s_rep = None
                    if c > 0:
                        s_rep = big.tile([P, K * TCMAX], bf16, tag="srep",
                                         name="srep")
                        ssrc = bass.AP(tensor=es_d.tensor,
                                       offset=es_d.offset + K * TCMAX,
                                       ap=[[0, P], [TCMAX, K], [1, TC]])
                        nc.scalar.dma_start(
                            out=s_rep[:, :].rearrange("p (k t) -> p k t", k=K)[:, :, 0:TC],
                            in_=ssrc)
                    it_ps = [pmm.tile([P, TCMAX], f32, tag="itps", name="itps")
                             for _ in range(NDO)]
                    for m in range(NDO):
                        for kt in range(NDO):
                            nc.tensor.matmul(
                                it_ps[m][:, 0:TC],
                                wi_sb[kt][:, m * P:(m + 1) * P],
                                xt_sb[kt][:, 0:TC],
                                start=(kt == 0), stop=(kt == NDO - 1))
                    it_sb = [wk.tile([P, TCMAX], bf16, tag=f"it{t}",
                                     name=f"it{t}") for t in range(NDO)]
                    for t in range(NDO):
                        nc.scalar.copy(out=it_sb[t][:, 0:TC],
                                       in_=it_ps[t][:, 0:TC])
                return it_sb, e_rep, s_rep

            fe = front_end(0)
            for c in range(NCHUNK):
                TC = CHUNKS[c]
                is_warm = (c == 0)
                it_sb, e_rep, s_rep = fe
                if c + 1 < NCHUNK:
                    fe = front_end(c + 1)

                # 5. per d-tile: z = e*i + scans on DVE (kept on one engine
                # so the z->scan chain never crosses engines); readout/tree
                # tails are split with Pool
                yt_sb = []
                y2_sb = []
                for t in range(NDO):
                    zm = zmp.tile([P, K * TCMAX], bf16, tag=f"zm{t}",
                                  name=f"zm{t}")
                    zm3 = zm[:, :].rearrange("p (k t) -> p k t", k=K)
                    it3 = bass.AP(tensor=it_sb[t].tensor,
                                  offset=it_sb[t].offset,
                                  ap=[it_sb[t].ap[0], [0, K], [1, TC]])
                    er3 = e_rep[:, :].rearrange("p (k t) -> p k t", k=K)
                    with tc.high_priority():
                        nc.vector.tensor_mul(out=zm3[:, :, 0:TC],
                                             in0=er3[:, :, 0:TC], in1=it3)
                    # per-k scans (DVE-only op); initial = extracted carry
                    for k in range(K):
                        col = oc_sb[t][:, k:k + 1]
                        dec = bass.AP(tensor=col.tensor, offset=col.offset,
                                      ap=[col.ap[0], [0, TC]])
                        init = 0.0 if c == 0 else carry[t][:, k:k + 1]
                        with tc.high_priority():
                            nc.vector.tensor_tensor_scan(
                                out=zm[:, k * TCMAX:k * TCMAX + TC],
                                data0=dec,
                                data1=zm[:, k * TCMAX:k * TCMAX + TC],
                                initial=init,
                                op0=OP.mult, op1=OP.add)
                    # save carries (last column of each k) before the
                    # in-place readout destroys them
                    if c < NCHUNK - 1:
                        lastcol = bass.AP(tensor=zm.tensor,
                                          offset=zm.offset + TC - 1,
                                          ap=[zm.ap[0], [TCMAX, K]])
                        with tc.high_priority():
                            nc.scalar.copy(out=carry[t][:, :], in_=lastcol)
                    if is_warm:
                        continue
                    # readout: w = s*m (in place), tree-reduce over k
                    sr3 = s_rep[:, :].rearrange("p (k t) -> p k t", k=K)
                    last = (c == NCHUNK - 1)
                    r_pool = (0, 1) if last else (2, 3)
                    t_pool = (0, 1) if last else (2, 3)
                    r_eng = nc.gpsimd if t in r_pool else nc.vector
                    r_eng.tensor_mul(out=zm3[:, :, 0:TC],
                                     in0=zm3[:, :, 0:TC],
                                     in1=sr3[:, :, 0:TC])
                    # k8 -> k4 -> k2 -> k1 (strided views over the k axis);
                    # the final add writes yt directly
                    t_eng = nc.gpsimd if t in t_pool else nc.vector
                    yt = wk.tile([P, TCMAX], bf16, tag=f"yt{t}", name=f"yt{t}")
                    for steps in (4, 2):
                        a = zm3[:, 0:steps, 0:TC]
                        b = zm3[:, steps:2 * steps, 0:TC]
                        t_eng.tensor_add(out=a, in0=a, in1=b)
                    t_eng.tensor_add(out=yt[:, 0:TC],
                                     in0=zm[:, 0:TC],
                                     in1=zm[:, TCMAX:TCMAX + TC])
                    # y^2 for variance (Act engine)
                    y2 = wk.tile([P, TCMAX], bf16, tag=f"y2{t}", name=f"y2{t}")
                    nc.scalar.activation(out=y2[:, 0:TC], in_=yt[:, 0:TC],
                                         func=AF.Square, scale=1.0)
                    yt_sb.append(yt)
                    y2_sb.append(y2)

                if is_warm:
                    continue

                # 6. LN stats via ones-matmuls: M = sum_d y, Q = sum_d y^2
                m_ps = pes.tile([1, TCMAX], f32, tag="mps", name="mps")
                q_ps = pes.tile([1, TCMAX], f32, tag="qps", name="qps")
                for t in range(NDO):
                    nc.tensor.matmul(m_ps[:, 0:TC], ones_sb[:, :],
                                     yt_sb[t][:, 0:TC],
                                     start=(t == 0), stop=(t == NDO - 1))
                for t in range(NDO):
                    nc.tensor.matmul(q_ps[:, 0:TC], ones_sb[:, :],
                                     y2_sb[t][:, 0:TC],
                                     start=(t == 0), stop=(t == NDO - 1))
                m_sb = wk.tile([1, TCMAX], f32, tag="msb", name="msb")
                q_sb = wk.tile([1, TCMAX], f32, tag="qsb", name="qsb")
                nc.scalar.copy(out=m_sb[:, 0:TC], in_=m_ps[:, 0:TC])
                nc.scalar.copy(out=q_sb[:, 0:TC], in_=q_ps[:, 0:TC])
                # row of -mu = M * (-1/512) for the rank-1 G correction (bf16)
                mneg = wk.tile([1, TCMAX], bf16, tag="mneg", name="mneg")
                nc.scalar.activation(out=mneg[:, 0:TC], in_=m_sb[:, 0:TC],
                                     func=AF.Copy, scale=-1.0 / D)
                # rsig row = 1/sqrt(Q/D - (M/D)^2 + eps)
                ntt = TC // P
                mu2 = wk.tile([1, TCMAX], f32, tag="mu2", name="mu2")
                nc.scalar.activation(out=mu2[:, 0:TC], in_=m_sb[:, 0:TC],
                                     func=AF.Square, scale=1.0 / D)
                var = wk.tile([1, TCMAX], f32, tag="var", name="var")
                nc.vector.scalar_tensor_tensor(out=var[:, 0:TC],
                                               in0=q_sb[:, 0:TC],
                                               scalar=1.0 / D,
                                               in1=mu2[:, 0:TC],
                                               op0=OP.mult, op1=OP.subtract)
                sig = wk.tile([1, TCMAX], f32, tag="sig", name="sig")
                nc.scalar.activation(out=sig[:, 0:TC], in_=var[:, 0:TC],
                                     func=AF.Sqrt, bias=eps_sb[0:1, :],
                                     scale=1.0)
                rsigrow = wk.tile([1, TCMAX], f32, tag="rsigrow", name="rsigrow")
                nc.vector.reciprocal(out=rsigrow[:, 0:TC], in_=sig[:, 0:TC])
                # transpose rsig row slices into columns for the epilogue
                rsig = wk.tile([P, 4], f32, tag="rsig", name="rsig")
                for tt in range(ntt):
                    r = rsigrow[0:1, tt * P:(tt + 1) * P]
                    nc.sync.dma_start(out=rsig[:, tt:tt + 1], in_=r)

                # 7. G = y^T @ Wo' + (-mu) x h + 1 x bow, epilogue, store
                for tt in range(ntt):
                    g_ps = pg.tile([P, D], f32, tag="gps", name="gps")
                    tsl = slice(tt * P, (tt + 1) * P)
                    for t in range(NDO):
                        nc.tensor.matmul(g_ps[:, :], yt_sb[t][:, tsl],
                                         wo_sb[t][:, :],
                                         start=(t == 0), stop=False)
                    nc.tensor.matmul(g_ps[:, :], mneg[:, tsl], h_sb[:, :],
                                     start=False, stop=False)
                    nc.tensor.matmul(g_ps[:, :], onerow[:, :], bow_sb[:, :],
                                     start=False, stop=True)
                    out_sb = wk.tile([P, D], f32, tag="outp", name="outp")
                    nc.scalar.activation(out=out_sb[:, :], in_=g_ps[:, :],
                                         func=AF.Copy, bias=0.0,
                                         scale=rsig[:, tt:tt + 1])
                    orow = coff[c] - W + tt * P
                    nc.sync.dma_start(out=yout[orow:orow + P, :],
                                      in_=out_sb[:, :])

    nc.compile()
    return nc


def _prep_inputs(x, W_i, W_e, W_s, o_param, ln_gamma, ln_beta, W_out):
    # stable logsigmoid: log sigmoid(w) = min(w,0) - log1p(exp(-|w|))
    o = np.exp(np.log1p(np.exp(-np.abs(o_param))) * (-1.0 / TAU)
               + np.minimum(o_param, 0.0) / TAU).astype(np.float32)
    wes = np.concatenate([W_e, W_s], axis=1).astype(np.float32)
    wo = (ln_gamma[:, None] * W_out).astype(np.float32)
    wo_h = wo.astype(ml_dtypes.bfloat16)
    hrow = wo_h.astype(np.float32).sum(axis=0, keepdims=True)
    bowr = (ln_beta @ W_out).astype(np.float32)[None, :]
    shared = {
        "wi": np.ascontiguousarray(W_i, np.float32),
        "wes": np.ascontiguousarray(wes),
        "oc": np.ascontiguousarray(o.T),
        "wo": np.ascontiguousarray(wo_h),
        "hrow": np.ascontiguousarray(hrow.astype(ml_dtypes.bfloat16)),
        "bowr": np.ascontiguousarray(bowr.astype(ml_dtypes.bfloat16)),
    }
    in_maps = []
    for core in range(8):
        b, h = core // 2, core % 2
        t0 = h * H
        lo = t0 - W
        if lo < 0:
            xs = np.concatenate(
                [np.zeros((W, D), np.float32), x[b, 0:t0 + H]], axis=0)
        else:
            xs = x[b, lo:t0 + H]
        m = dict(shared)
        m["xt"] = np.ascontiguousarray(xs.T, np.float32)
        in_maps.append(m)
    return in_maps


def kernel(x, W_i, W_e, W_s, o_param, ln_gamma, ln_beta, W_out):
    from concourse.bass_utils import run_bass_kernel_spmd

    if "nc" not in _CACHE:
        _CACHE["nc"] = _build()
    nc = _CACHE["nc"]

    in_maps = _prep_inputs(np.asarray(x, np.float32), np.asarray(W_i),
                           np.asarray(W_e), np.asarray(W_s),
                           np.asarray(o_param), np.asarray(ln_gamma),
                           np.asarray(ln_beta), np.asarray(W_out))
    res = run_bass_kernel_spmd(nc, in_maps, core_ids=list(range(8)))
    out = np.empty((B, N, D), np.float32)
    for core in range(8):
        b, h = core // 2, core % 2
        out[b, h * H:(h + 1) * H] = res.results[core]["yout"]
    return out


# revision 26
# speedup vs baseline: 1.1025x; 1.1025x over previous
"""Trainium2 Bass kernel for nn_EosLayer (gated linear-attention recurrence).

Sharding: 8 cores = 4 batches x 2 sequence halves. Each core processes
T = 256 (warmup) + 2048 (output) timesteps of one batch. The warmup window
replaces cross-core state passing: the per-(k,d) decay o < 0.97 makes
history older than 256 steps contribute < 3e-3 relative, well under the
bf16 noise floor of this implementation.

Per-core layout is d-major (d on partitions, time on the free dim).
Chunk grid: [256 (pure warmup), 512, 512, 512, 512].

Engine assignment (driven by the TimelineSim cost model):
  - PE: i/e/s projections (f32r), LN-stat ones-matmuls, output GEMM with
    rank-1 corrections for -mu*h and beta@W_out (all bf16).
  - DVE: all elementwise tensor-tensor work in bf16 (z = e*i, w = s*m,
    k-tree-reduction) at the 2x 16-bit rate, plus LN row math.
  - Pool: the per-(k,d-tile) hardware scans (its best cost ratio op);
    a few scan sets go to DVE for balance.
  - Act: PSUM evacuations (downcast to bf16), y^2 squares, final scale.
  - DMA: e/s partition-replication via DRAM bounce (bf16 halves traffic).

The scan keeps fp32 internal state (hardware guarantee), reads bf16 z,
writes bf16 m; decay columns stay f32 (bf16 decay would bias every step).
Chunk carries are bf16 views of the previous chunk's scan output used
directly as the `initial` operand.
"""

import numpy as np
import ml_dtypes

D = 512
K = 8
TAU = 16.0
EPS = 1e-5
B = 4
N = 4096
H = N // 2          # output rows per core
W = 256             # warmup rows
T = W + H           # 2304 rows processed per core
CHUNKS = [256, 512, 512, 512, 512]   # chunk 0 is pure warmup
NDO = 4             # d-tiles of 128 partitions
P = 128
TCMAX = 512

_CACHE = {}


def _build():
    import concourse.bass as bass
    import concourse.mybir as mybir
    import concourse.tile as tile
    from concourse.bacc import Bacc

    f32 = mybir.dt.float32
    f32r = mybir.dt.float32r
    bf16 = mybir.dt.bfloat16
    AF = mybir.ActivationFunctionType
    OP = mybir.AluOpType

    nc = Bacc("TRN2", target_bir_lowering=False, debug=False,
              enable_asserts=False, num_devices=8)

    # per-core input (pre-transposed x slice), shared weight/const inputs
    xt = nc.dram_tensor("xt", (D, T), f32r, kind="ExternalInput")
    wi = nc.dram_tensor("wi", (D, D), f32r, kind="ExternalInput")
    wes = nc.dram_tensor("wes", (D, 2 * K), f32r, kind="ExternalInput")
    oc = nc.dram_tensor("oc", (D, K), f32, kind="ExternalInput")      # o.T
    wo = nc.dram_tensor("wo", (D, D), bf16, kind="ExternalInput")     # gamma-folded
    hrow = nc.dram_tensor("hrow", (1, D), bf16, kind="ExternalInput")  # colsum(wo)
    bowr = nc.dram_tensor("bowr", (1, D), bf16, kind="ExternalInput")  # beta @ W_out
    yout = nc.dram_tensor("yout", (H, D), f32, kind="ExternalOutput")

    NCHUNK = len(CHUNKS)
    coff = [sum(CHUNKS[:i]) for i in range(NCHUNK)]   # chunk start columns

    with tile.TileContext(nc) as tc:
        with tc.tile_pool(name="const", bufs=1) as cst, \
             tc.tile_pool(name="work", bufs=2) as wk, \
             tc.tile_pool(name="xtp", bufs=1) as xtp, \
             tc.tile_pool(name="big", bufs=2) as big, \
             tc.tile_pool(name="zmp", bufs=3) as zmp, \
             tc.tile_pool(name="pmm", bufs=4, space="PSUM") as pmm, \
             tc.tile_pool(name="pes", bufs=1, space="PSUM") as pes, \
             tc.tile_pool(name="pg", bufs=1, space="PSUM") as pg, \
             tc.tile_pool(name="dr", bufs=2, space="DRAM") as dr:

            # ---- constants (loaded once) ----
            wi_sb = [cst.tile([P, D], f32r, tag=f"wi{t}", name=f"wi{t}") for t in range(NDO)]
            wes_sb = [cst.tile([P, 2 * K], f32r, tag=f"wes{t}", name=f"wes{t}") for t in range(NDO)]
            oc_sb = [cst.tile([P, K], f32, tag=f"oc{t}", name=f"oc{t}") for t in range(NDO)]
            wo_sb = [cst.tile([P, D], bf16, tag=f"wo{t}", name=f"wo{t}") for t in range(NDO)]
            # wes/wi feed chunk 0's projections immediately (SP queue, ahead
            # of the xt stream); oc/wo/h/bow are needed later and issue from
            # the Act queue so they don't serialize the SP queue.
            for t in range(NDO):
                sl = slice(t * P, (t + 1) * P)
                nc.sync.dma_start(out=wes_sb[t], in_=wes[sl, :])
                nc.sync.dma_start(out=wi_sb[t], in_=wi[sl, :])
            for t in range(NDO):
                sl = slice(t * P, (t + 1) * P)
                nc.gpsimd.dma_start(out=oc_sb[t], in_=oc[sl, :])
                nc.gpsimd.dma_start(out=wo_sb[t], in_=wo[sl, :])
            h_sb = cst.tile([1, D], bf16, tag="h", name="h")
            nc.gpsimd.dma_start(out=h_sb, in_=hrow[:, :])
            bow_sb = cst.tile([1, D], bf16, tag="bow", name="bow")
            nc.gpsimd.dma_start(out=bow_sb, in_=bowr[:, :])
            ones_sb = cst.tile([P, 1], bf16, tag="ones", name="ones")
            nc.vector.memset(ones_sb, 1.0)
            onerow = cst.tile([1, P], bf16, tag="onerow", name="onerow")
            nc.vector.memset(onerow, 1.0)
            eps_sb = cst.tile([P, 1], f32, tag="eps", name="eps")
            nc.vector.memset(eps_sb, EPS)

            # per-d-tile scan carries (extracted before the in-place readout)
            carry = [cst.tile([P, K], f32, tag=f"carry{t}", name=f"carry{t}")
                     for t in range(NDO)]

            def front_end(c):
                """Steps 1-4 of chunk c: x load, projections, evacuations,
                e/s partition-replication. Emitted one chunk ahead so PE's
                projections are queued before the previous chunk's LN/GEMM
                tail (avoids head-of-line chunk serialization)."""
                TC = CHUNKS[c]
                csl = slice(coff[c], coff[c] + TC)
                with tc.high_priority():
                    xt_sb = [xtp.tile([P, TCMAX], f32r, tag=f"xt{t}",
                                      name=f"xt{t}") for t in range(NDO)]
                    for t in range(NDO):
                        nc.sync.dma_start(out=xt_sb[t][:, 0:TC],
                                          in_=xt[t * P:(t + 1) * P, csl])
                    es_ps = pes.tile([2 * K, TCMAX], f32, tag="esps", name="esps")
                    for kt in range(NDO):
                        nc.tensor.matmul(es_ps[:, 0:TC], wes_sb[kt][:, :],
                                         xt_sb[kt][:, 0:TC],
                                         start=(kt == 0), stop=(kt == NDO - 1))
                    es_sb = wk.tile([2 * K, TCMAX], bf16, tag="es", name="es")
                    nc.scalar.copy(out=es_sb[:, 0:TC], in_=es_ps[:, 0:TC])
                    es_d = dr.tile([2 * K, TCMAX], bf16, tag="esd", name="esd")
                    nc.scalar.dma_start(out=es_d[:, 0:TC], in_=es_sb[:, 0:TC])
                    e_rep = big.tile([P, K * TCMAX], bf16, tag="erep", name="erep")
                    esrc = bass.AP(tensor=es_d.tensor, offset=es_d.offset,
                                   ap=[[0, P], [TCMAX, K], [1, TC]])
                    nc.scalar.dma_start(
                        out=e_rep[:, :].rearrange("p (k t) -> p k t", k=K)[:, :, 0:TC],
                        in_=esrc)
                    s_rep = None
                    if c > 0:
                        s_rep = big.tile([P, K * TCMAX], bf16, tag="srep",
                                         name="srep")
                        ssrc = bass.AP(tensor=es_d.tensor,
                                       offset=es_d.offset + K * TCMAX,
                                       ap=[[0, P], [TCMAX, K], [1, TC]])
                        nc.scalar.dma_start(
                            out=s_rep[:, :].rearrange("p (k t) -> p k t", k=K)[:, :, 0:TC],
                            in_=ssrc)
                    it_ps = [pmm.tile([P, TCMAX], f32, tag="itps", name="itps")
                             for _ in range(NDO)]
                    for m in range(NDO):
                        for kt in range(NDO):
                            nc.tensor.matmul(
                                it_ps[m][:, 0:TC],
                                wi_sb[kt][:, m * P:(m + 1) * P],
                                xt_sb[kt][:, 0:TC],
                                start=(kt == 0), stop=(kt == NDO - 1))
                    it_sb = [wk.tile([P, TCMAX], bf16, tag=f"it{t}",
                                     name=f"it{t}") for t in range(NDO)]
                    for t in range(NDO):
                        nc.scalar.copy(out=it_sb[t][:, 0:TC],
                                       in_=it_ps[t][:, 0:TC])
                return it_sb, e_rep, s_rep

            fe = front_end(0)
            for c in range(NCHUNK):
                TC = CHUNKS[c]
                is_warm = (c == 0)
                it_sb, e_rep, s_rep = fe
                if c + 1 < NCHUNK:
                    fe = front_end(c + 1)

                # 5. per d-tile: z = e*i + scans on DVE (kept on one engine
                # so the z->scan chain never crosses engines); readout/tree
                # tails are split with Pool
                yt_sb = []
                y2_sb = []
                for t in range(NDO):
                    zm = zmp.tile([P, K * TCMAX], bf16, tag=f"zm{t}",
                                  name=f"zm{t}")
                    zm3 = zm[:, :].rearrange("p (k t) -> p k t", k=K)
                    it3 = bass.AP(tensor=it_sb[t].tensor,
                                  offset=it_sb[t].offset,
                                  ap=[it_sb[t].ap[0], [0, K], [1, TC]])
                    er3 = e_rep[:, :].rearrange("p (k t) -> p k t", k=K)
                    with tc.high_priority():
                        nc.vector.tensor_mul(out=zm3[:, :, 0:TC],
                                             in0=er3[:, :, 0:TC], in1=it3)
                    # per-k scans (DVE-only op); initial = extracted carry
                    for k in range(K):
                        col = oc_sb[t][:, k:k + 1]
                        dec = bass.AP(tensor=col.tensor, offset=col.offset,
                                      ap=[col.ap[0], [0, TC]])
                        init = 0.0 if c == 0 else carry[t][:, k:k + 1]
                        with tc.high_priority():
                            nc.vector.tensor_tensor_scan(
                                out=zm[:, k * TCMAX:k * TCMAX + TC],
                                data0=dec,
                                data1=zm[:, k * TCMAX:k * TCMAX + TC],
                                initial=init,
                                op0=OP.mult, op1=OP.add)
                    # save carries (last column of each k) before the
                    # in-place readout destroys them
                    if c < NCHUNK - 1:
                        lastcol = bass.AP(tensor=zm.tensor,
                                          offset=zm.offset + TC - 1,
                                          ap=[zm.ap[0], [TCMAX, K]])
                        with tc.high_priority():
                            nc.scalar.copy(out=carry[t][:, :], in_=lastcol)
                    if is_warm:
                        continue
                    # readout: w = s*m (in place), tree-reduce over k
                    sr3 = s_rep[:, :].rearrange("p (k t) -> p k t", k=K)
                    last = (c == NCHUNK - 1)
                    r_pool = (0, 1) if last else (2, 3)
                    t_pool = (0,) if last else (3,)
                    r_eng = nc.gpsimd if t in r_pool else nc.vector
                    r_eng.tensor_mul(out=zm3[:, :, 0:TC],
                                     in0=zm3[:, :, 0:TC],
                                     in1=sr3[:, :, 0:TC])
                    # k8 -> k4 -> k2 -> k1 (strided views over the k axis);
                    # the final add writes yt directly
                    t_eng = nc.gpsimd if t in t_pool else nc.vector
                    yt = wk.tile([P, TCMAX], bf16, tag=f"yt{t}", name=f"yt{t}")
                    for steps in (4, 2):
                        a = zm3[:, 0:steps, 0:TC]
                        b = zm3[:, steps:2 * steps, 0:TC]
                        t_eng.tensor_add(out=a, in0=a, in1=b)
                    t_eng.tensor_add(out=yt[:, 0:TC],
                                     in0=zm[:, 0:TC],
                                     in1=zm[:, TCMAX:TCMAX + TC])
                    # y^2 for variance (Act engine)
                    y2 = wk.tile([P, TCMAX], bf16, tag=f"y2{t}", name=f"y2{t}")
                    nc.scalar.activation(out=y2[:, 0:TC], in_=yt[:, 0:TC],
                                         func=AF.Square, scale=1.0)
                    yt_sb.append(yt)
                    y2_sb.append(y2)

                if is_warm:
                    continue

                # 6. LN stats via ones-matmuls: M = sum_d y, Q = sum_d y^2
                m_ps = pes.tile([1, TCMAX], f32, tag="mps", name="mps")
                q_ps = pes.tile([1, TCMAX], f32, tag="qps", name="qps")
                for t in range(NDO):
                    nc.tensor.matmul(m_ps[:, 0:TC], ones_sb[:, :],
                                     yt_sb[t][:, 0:TC],
                                     start=(t == 0), stop=(t == NDO - 1))
                for t in range(NDO):
                    nc.tensor.matmul(q_ps[:, 0:TC], ones_sb[:, :],
                                     y2_sb[t][:, 0:TC],
                                     start=(t == 0), stop=(t == NDO - 1))
                m_sb = wk.tile([1, TCMAX], f32, tag="msb", name="msb")
                q_sb = wk.tile([1, TCMAX], f32, tag="qsb", name="qsb")
                nc.scalar.copy(out=m_sb[:, 0:TC], in_=m_ps[:, 0:TC])
                nc.scalar.copy(out=q_sb[:, 0:TC], in_=q_ps[:, 0:TC])
                # row of -mu = M * (-1/512) for the rank-1 G correction (bf16)
                mneg = wk.tile([1, TCMAX], bf16, tag="mneg", name="mneg")
                nc.scalar.activation(out=mneg[:, 0:TC], in_=m_sb[:, 0:TC],
                                     func=AF.Copy, scale=-1.0 / D)
                # rsig row = 1/sqrt(Q/D - (M/D)^2 + eps)
                ntt = TC // P
                mu2 = wk.tile([1, TCMAX], f32, tag="mu2", name="mu2")
                nc.scalar.activation(out=mu2[:, 0:TC], in_=m_sb[:, 0:TC],
                                     func=AF.Square, scale=1.0 / D)
                var = wk.tile([1, TCMAX], f32, tag="var", name="var")
                nc.vector.scalar_tensor_tensor(out=var[:, 0:TC],
                                               in0=q_sb[:, 0:TC],
                                               scalar=1.0 / D,
                                               in1=mu2[:, 0:TC],
                                               op0=OP.mult, op1=OP.subtract)
                sig = wk.tile([1, TCMAX], f32, tag="sig", name="sig")
                nc.scalar.activation(out=sig[:, 0:TC], in_=var[:, 0:TC],
                                     func=AF.Sqrt, bias=eps_sb[0:1, :],
                                     scale=1.0)
                rsigrow = wk.tile([1, TCMAX], f32, tag="rsigrow", name="rsigrow")
                nc.vector.reciprocal(out=rsigrow[:, 0:TC], in_=sig[:, 0:TC])
                # transpose rsig row slices into columns for the epilogue
                rsig = wk.tile([P, 4], f32, tag="rsig", name="rsig")
                for tt in range(ntt):
                    r = rsigrow[0:1, tt * P:(tt + 1) * P]
                    nc.sync.dma_start(out=rsig[:, tt:tt + 1], in_=r)

                # 7. G = y^T @ Wo' + (-mu) x h + 1 x bow, epilogue, store
                for tt in range(ntt):
                    g_ps = pg.tile([P, D], f32, tag="gps", name="gps")
                    tsl = slice(tt * P, (tt + 1) * P)
                    for t in range(NDO):
                        nc.tensor.matmul(g_ps[:, :], yt_sb[t][:, tsl],
                                         wo_sb[t][:, :],
                                         start=(t == 0), stop=False)
                    nc.tensor.matmul(g_ps[:, :], mneg[:, tsl], h_sb[:, :],
                                     start=False, stop=False)
                    nc.tensor.matmul(g_ps[:, :], onerow[:, :], bow_sb[:, :],
                                     start=False, stop=True)
                    out_sb = wk.tile([P, D], f32, tag="outp", name="outp")
                    nc.scalar.activation(out=out_sb[:, :], in_=g_ps[:, :],
                                         func=AF.Copy, bias=0.0,
                                         scale=rsig[:, tt:tt + 1])
                    orow = coff[c] - W + tt * P
                    nc.sync.dma_start(out=yout[orow:orow + P, :],
                                      in_=out_sb[:, :])

    nc.compile()
    return nc


def _prep_inputs(x, W_i, W_e, W_s, o_param, ln_gamma, ln_beta, W_out):
    # stable logsigmoid: log sigmoid(w) = min(w,0) - log1p(exp(-|w|))
    o = np.exp(np.log1p(np.exp(-np.abs(o_param))) * (-1.0 / TAU)
               + np.minimum(o_param, 0.0) / TAU).astype(np.float32)
    wes = np.concatenate([W_e, W_s], axis=1).astype(np.float32)
    wo = (ln_gamma[:, None] * W_out).astype(np.float32)
    wo_h = wo.astype(ml_dtypes.bfloat16)
    hrow = wo_h.astype(np.float32).sum(axis=0, keepdims=True)
    bowr = (ln_beta @ W_out).astype(np.float32)[None, :]
    shared = {
        "wi": np.ascontiguousarray(W_i, np.float32),
        "wes": np.ascontiguousarray(wes),
        "oc": np.ascontiguousarray(o.T),
        "wo": np.ascontiguousarray(wo_h),
        "hrow": np.ascontiguousarray(hrow.astype(ml_dtypes.bfloat16)),
        "bowr": np.ascontiguousarray(bowr.astype(ml_dtypes.bfloat16)),
    }
    in_maps = []
    for core in range(8):
        b, h = core // 2, core % 2
        t0 = h * H
        lo = t0 - W
        if lo < 0:
            xs = np.concatenate(
                [np.zeros((W, D), np.float32), x[b, 0:t0 + H]], axis=0)
        else:
            xs = x[b, lo:t0 + H]
        m = dict(shared)
        m["xt"] = np.ascontiguousarray(xs.T, np.float32)
        in_maps.append(m)
    return in_maps


def kernel(x, W_i, W_e, W_s, o_param, ln_gamma, ln_beta, W_out):
    from concourse.bass_utils import run_bass_kernel_spmd

    if "nc" not in _CACHE:
        _CACHE["nc"] = _build()
    nc = _CACHE["nc"]

    in_maps = _prep_inputs(np.asarray(x, np.float32), np.asarray(W_i),
                           np.asarray(W_e), np.asarray(W_s),
                           np.asarray(o_param), np.asarray(ln_gamma),
                           np.asarray(ln_beta), np.asarray(W_out))
    res = run_bass_kernel_spmd(nc, in_maps, core_ids=list(range(8)))
    out = np.empty((B, N, D), np.float32)
    for core in range(8):
        b, h = core // 2, core % 2
        out[b, h * H:(h + 1) * H] = res.results[core]["yout"]
    return out
